# revision 21
# baseline (speedup 1.0000x reference)
"""Trainium2 Bass kernel for a margin-softmax cross-entropy loss.

Baseline (62882ns): see git-less problem dir. Restored from session read.
"""

from contextlib import ExitStack

import ml_dtypes
import numpy as np

import concourse.bass as bass
import concourse.tile as tile
from concourse import bacc, mybir
from concourse.bass_utils import run_bass_kernel_spmd

B = 4096
D = 512
C = 10575
NCORES = 8
CS_BASE = 1322        # real classes on cores 0..6; core 7 gets 1321
CSH = 1328            # padded per-core class count
# class-tile widths/offsets: c0+c1 fill psA (2 PSUM banks, ScalarE's share),
# c2 fills psB (1 bank, VectorE's share). Separate tiles keep the two
# engines' reads on disjoint PSUM banks - the Tile dependency tracker works
# at bank granularity, and a shared bank serializes the readers.
CW = (512, 304, 512)
CO = (0, 512, 816)
KP = 2                # fp8 DoubleRow k-pairs (256 contraction each)
BT = B // 128         # 32 batch tiles
BSH = B // NCORES     # 512 rows of label logits per core
WSCALE = 64.0         # fp8 pre-scale for w (subnormal-range fix), undone in exp

XA = 816              # exp cols on ScalarE (accum_out row-sum) = psA width
XD = CSH - XA         # fast-exp cols on VectorE (512) = psB width
XDH = XD // 2         # 256: half width for the fused fold+reduce

# Schraudolph fast-exp in f16: bits of exp(l) ~ round(l * 2^10/ln2 + M0H).
# M0H tuned for zero mean bias of sum(exp) under the empirical logit
# distribution (std ~0.303); K1S folds in the 1/WSCALE PSUM pre-scale.
K1H = 1477.3197218702985
M0H = 15301.739746
K1S = float(np.float32(K1H / WSCALE))

BF16 = mybir.dt.bfloat16
F16 = mybir.dt.float16
FP8 = mybir.dt.float8e4
F32 = mybir.dt.float32
I16 = mybir.dt.int16

_CACHE = {}


def _build_nc():
    nc = bacc.Bacc("TRN2", debug=False, target_bir_lowering=False)

    # critical pack: [wS_k0|fT0_k0 | wS_k1|fT0_k1] per partition
    HK = 2 * CSH + 1024
    H1 = KP * HK
    head1 = nc.dram_tensor("head1", [128, H1], FP8, kind="ExternalInput").ap()
    fTr = nc.dram_tensor("fTr", [KP, 128, 2, B - 512], FP8, kind="ExternalInput").ap()
    outS = nc.dram_tensor("outS", [128, 2 * BT], F32, kind="ExternalOutput").ap()

    with tile.TileContext(nc) as tc, ExitStack() as ctx:
        consts = ctx.enter_context(tc.tile_pool(name="consts", bufs=1))
        psums_a = ctx.enter_context(tc.tile_pool(name="psums_a", bufs=2, space="PSUM"))
        psums_b = ctx.enter_context(tc.tile_pool(name="psums_b", bufs=2, space="PSUM"))
        psumw = ctx.enter_context(tc.tile_pool(name="psumw", bufs=1, space="PSUM"))
        epool = ctx.enter_context(tc.tile_pool(name="epool", bufs=3))
        hpool = ctx.enter_context(tc.tile_pool(name="hpool", bufs=2))
        ypool = ctx.enter_context(tc.tile_pool(name="ypool", bufs=2))
        outs = ctx.enter_context(tc.tile_pool(name="outs", bufs=1))

        # input DMAs first, chained on the one SP HWDGE ring in priority
        # order: the queue drains in issue order, so head-A (everything
        # strip-0's k0 matmuls need) hits the wire first.
        CUT_A = 2 * CW[0] + 1024          # wS_k0_c0 + fT0_k0
        CUT_B = CUT_A + 2 * (CW[1] + CW[2])   # + wS_k0_c1, wS_k0_c2 (k0 half end)
        OFF_K0 = {0: 0, 1: CUT_A, 2: CUT_A + 2 * CW[1]}
        OFF_K1 = {0: CUT_B, 1: CUT_B + 2 * CW[0],
                  2: CUT_B + 2 * (CW[0] + CW[1])}
        FT1_OFF = CUT_B + 2 * CSH
        FSPLIT = 1024                     # fTr chunk split (strips 4-11 | 12-31)

        head1_sb = consts.tile([128, H1], FP8, tag="head1")
        nc.sync.dma_start(out=head1_sb[:, 0:CUT_A], in_=head1[:, 0:CUT_A])
        nc.sync.dma_start(out=head1_sb[:, CUT_A:CUT_B], in_=head1[:, CUT_A:CUT_B])
        nc.sync.dma_start(out=head1_sb[:, CUT_B:H1], in_=head1[:, CUT_B:H1])
        fTr_sb = []
        for k in range(KP):
            t = consts.tile([128, 2, B - 512], FP8, tag=f"fTr{k}")
            fTr_sb.append(t)
        for k in range(KP):
            nc.sync.dma_start(
                out=fTr_sb[k][:, :, 0:FSPLIT], in_=fTr[k][:, :, 0:FSPLIT])
        for k in range(KP):
            nc.sync.dma_start(
                out=fTr_sb[k][:, :, FSPLIT:B - 512],
                in_=fTr[k][:, :, FSPLIT:B - 512])

        # PE HAM warm-up
        warm = consts.tile([128, 384], BF16, tag="warm")
        nc.gpsimd.memset(warm[:], 0.0)
        pwt = psumw.tile([128, 384], F32, tag="pw")
        for _ in range(10):
            nc.tensor.matmul(out=pwt[:], lhsT=warm[:, 0:128],
                             rhs=warm[:], start=True, stop=True)
        eprime = consts.tile([128, 1], F16, tag="eprime")
        nc.scalar.activation(
            out=eprime[:], in_=warm[:, 0:1].bitcast(F16),
            func=mybir.ActivationFunctionType.Exp, scale=1.0,
        )

        wS_sb = [
            {c: head1_sb[:, off:off + 2 * CW[c]].rearrange(
                "p (i n) -> p i n", i=2)
             for c, off in offs.items()}
            for offs in (OFF_K0, OFF_K1)
        ]
        fT0_sb = [
            head1_sb[:, 2 * 512:CUT_A].rearrange("p (i n) -> p i n", i=2),
            head1_sb[:, FT1_OFF:FT1_OFF + 1024].rearrange(
                "p (i n) -> p i n", i=2),
        ]

        # main GEMM; per-strip exp+row-sum split BY COLUMN across engines:
        # ScalarE exp+accum over [0:XA), VectorE f16 fast-exp + fused
        # fold+accum over [XA:CSH)
        st = outs.tile([128, 2 * BT], F32, tag="st")

        def emit_matmuls(b, psa, psb, k):
            for c in range(len(CW)):
                lhsT = (fT0_sb[k][:, :, b * 128:(b + 1) * 128]
                        if b < 4 else
                        fTr_sb[k][:, :, (b - 4) * 128:(b - 3) * 128])
                out_ap = (psa[:, CO[c]:CO[c] + CW[c]] if c < 2
                          else psb[:, 0:CW[2]])
                nc.tensor.matmul(
                    out=out_ap,
                    lhsT=lhsT,
                    rhs=wS_sb[k][c][:],
                    start=(k == 0),
                    stop=(k == KP - 1),
                    perf_mode=mybir.MatmulPerfMode.DoubleRow,
                )

        def emit_consumers(b, psa, psb):
            # ScalarE: exp over [0:XA) with the accumulator doing the row-sum
            e = epool.tile([128, XA], F16, tag="e")
            nc.scalar.activation(
                out=e[:], in_=psa[:],
                func=mybir.ActivationFunctionType.Exp,
                scale=float(1.0 / WSCALE),
                accum_out=st[:, b:b + 1],
            )
            # VectorE: f16 fast-exp over [XA:CSH), then one fused fold+accum
            y = ypool.tile([128, XD], I16, tag="y")
            nc.vector.tensor_scalar(
                out=y[:], in0=psb[:], scalar1=K1S, scalar2=float(M0H),
                op0=mybir.AluOpType.mult, op1=mybir.AluOpType.add,
            )
            hh = hpool.tile([128, XDH], F16, tag="h1")
            nc.vector.scalar_tensor_tensor(
                out=hh[:], in0=y[:, 0:XDH].bitcast(F16), scalar=1.0,
                in1=y[:, XDH:XD].bitcast(F16),
                op0=mybir.AluOpType.mult, op1=mybir.AluOpType.add,
                accum_out=st[:, BT + b:BT + b + 1],
            )

        # strips 0-1: both k0 pair-halves emitted before the k1 halves, so
        # the in-order PE can chew through head-A work while head-B is still
        # on the wire
        psa0 = psums_a.tile([128, XA], F32, tag="psa")
        psb0 = psums_b.tile([128, XD], F32, tag="psb")
        psa1 = psums_a.tile([128, XA], F32, tag="psa")
        psb1 = psums_b.tile([128, XD], F32, tag="psb")
        pro = {0: (psa0, psb0), 1: (psa1, psb1)}
        for k, bb in ((0, 0), (0, 1), (1, 0), (1, 1)):
            emit_matmuls(bb, pro[bb][0], pro[bb][1], k)
        for bb in (0, 1):
            emit_consumers(bb, pro[bb][0], pro[bb][1])

        for b in range(2, BT):
            psa = psums_a.tile([128, XA], F32, tag="psa")
            psb = psums_b.tile([128, XD], F32, tag="psb")
            for k in range(KP):
                emit_matmuls(b, psa, psb, k)
            emit_consumers(b, psa, psb)
            if b == 15:
                nc.sync.dma_start(out=outS[:, 0:16], in_=st[:, 0:16])
            if b == BT - 4:
                nc.sync.dma_start(
                    out=outS[:, 16:BT - 4], in_=st[:, 16:BT - 4])
        # final chunk from the Activation ring: the last ACT accums plus the
        # whole fast-exp plane
        nc.scalar.dma_start(out=outS[:, BT - 4:], in_=st[:, BT - 4:])

    nc.compile()
    return nc


def _core_sizes():
    sizes = [CS_BASE] * (NCORES - 1) + [C - CS_BASE * (NCORES - 1)]
    starts = np.concatenate([[0], np.cumsum(sizes)[:-1]]).astype(np.int64)
    return np.array(sizes, dtype=np.int64), starts


def _prepare_inputs(feats, labels, w):
    sizes, starts = _core_sizes()

    in_maps = []
    fp8_feats = feats.astype(ml_dtypes.float8_e4m3)
    for p in range(NCORES):
        frolled = np.roll(fp8_feats, -p * BSH, axis=0)
        fT_host = np.ascontiguousarray(
            frolled.reshape(B, KP, 2, 128).transpose(1, 3, 2, 0)
        )
        fTr_host = np.ascontiguousarray(fT_host[:, :, :, 512:])
        c0, sz = int(starts[p]), int(sizes[p])
        wp = np.zeros((D, CSH), dtype=np.float32)
        wp[:, :sz] = w[:, c0:c0 + sz] * WSCALE
        wS_host = np.ascontiguousarray(
            wp.reshape(KP, 2, 128, CSH).transpose(0, 2, 1, 3)
        ).astype(ml_dtypes.float8_e4m3)

        def ctile(k, c):
            return np.ascontiguousarray(
                wS_host[k][:, :, CO[c]:CO[c] + CW[c]]).reshape(128, -1)

        head1_host = np.concatenate(
            [ctile(0, 0),
             np.ascontiguousarray(fT_host[0][:, :, 0:512]).reshape(128, -1),
             ctile(0, 1), ctile(0, 2),
             ctile(1, 0), ctile(1, 1), ctile(1, 2),
             np.ascontiguousarray(fT_host[1][:, :, 0:512]).reshape(128, -1)],
            axis=1,
        )
        in_maps.append({
            "head1": np.ascontiguousarray(head1_host),
            "fTr": fTr_host,
        })
    return in_maps


def _run(in_maps, trace=False):
    if "nc" not in _CACHE:
        _CACHE["nc"] = _build_nc()
    nc = _CACHE["nc"]
    return run_bass_kernel_spmd(
        nc, in_maps, core_ids=list(range(NCORES)), trace=trace
    )


def _fastexp_host(ps_vals):
    """Replica of the device f16 fast-exp for f32 PSUM values:
    f16 bits = rint(f32(f32(ps * K1S) + M0H)), read back as f16 floats."""
    x = np.asarray(ps_vals, dtype=np.float32)
    y = np.float32(x * np.float32(K1S)) + np.float32(M0H)
    return np.rint(y).astype(np.int16).view(np.float16).astype(np.float64)


def kernel(feats, labels, centers, counts, w, _trace=False, _ret_res=False):
    feats = np.asarray(feats, dtype=np.float32)
    labels_i = np.asarray(labels).astype(np.int64)
    centers = np.asarray(centers, dtype=np.float32)
    counts = np.asarray(counts, dtype=np.float32)
    w = np.asarray(w, dtype=np.float32)

    in_maps = _prepare_inputs(feats, labels_i, w)
    res = _run(in_maps, trace=_trace)

    sizes, starts = _core_sizes()

    means = (centers / counts[:, None]).astype(np.float32)
    nrm = np.sqrt((means.astype(np.float32) ** 2).sum(axis=1, keepdims=True))
    mn = (means / nrm).astype(np.float32)
    dsq = (mn.astype(np.float64) ** 2).sum(axis=1)       # [C]
    d = dsq[labels_i]                                    # [B]

    f8 = feats.astype(ml_dtypes.float8_e4m3).astype(np.float64)      # [B, D]
    w8 = (w * WSCALE).astype(ml_dtypes.float8_e4m3).astype(np.float64)
    t_ps = np.einsum("bd,bd->b", f8, w8[:, labels_i].T)              # [B]
    t = t_ps / WSCALE

    # pad cols (local idx >= sizes[p] >= XA) all take the fast-exp path:
    # each contributes fastexp16(0) per strip row
    fastexp0 = float(_fastexp_host(np.zeros(1))[0])

    S_tot = np.zeros(B, dtype=np.float64)
    for p in range(NCORES):
        # outS[q, b] is rolled row b*128 + q = original row (b*128+q+p*BSH)%B
        sp = res.results[p]["outS"].astype(np.float64)   # [128, 2*BT]
        sb = sp[:, :BT] + sp[:, BT:]                     # ACT + DVE shares
        S_p = sb.T.reshape(B)                            # rolled rows
        pad_p = float(CSH - sizes[p])
        S_p = S_p - pad_p * fastexp0
        S_tot += np.roll(S_p, p * BSH)

    # subtract the device's own label-column contribution: row i's label
    # class lives in shard p*, in the ACT range if its local column < XA
    p_star = np.minimum(labels_i // CS_BASE, NCORES - 1)
    c_local = labels_i - starts[p_star]
    lab_dev = np.where(
        c_local < XA,
        np.exp(t),
        _fastexp_host(t_ps),
    )
    z = S_tot - lab_dev + np.exp(t + d)
    nll = np.log(z) - (t + d)
    loss = np.float32(nll.mean())
    out = np.array(loss, dtype=np.float32)
    if _ret_res:
        return out, res
    return out


# revision 23
# speedup vs baseline: 1.0285x; 1.0285x over previous
"""Trainium2 Bass kernel for a margin-softmax cross-entropy loss.

Baseline (62882ns): see git-less problem dir. Restored from session read.
"""

from contextlib import ExitStack

import ml_dtypes
import numpy as np

import concourse.bass as bass
import concourse.tile as tile
from concourse import bacc, mybir
from concourse.bass_utils import run_bass_kernel_spmd

B = 4096
D = 512
C = 10575
NCORES = 8
CS_BASE = 1322        # real classes on cores 0..6; core 7 gets 1321
CSH = 1328            # padded per-core class count
# class-tile widths/offsets: c0+c1 fill psA (2 PSUM banks, ScalarE's share),
# c2 fills psB (1 bank, VectorE's share). Separate tiles keep the two
# engines' reads on disjoint PSUM banks - the Tile dependency tracker works
# at bank granularity, and a shared bank serializes the readers.
CW = (512, 304, 512)
CO = (0, 512, 816)
KP = 2                # fp8 DoubleRow k-pairs (256 contraction each)
BT = B // 128         # 32 batch tiles
BSH = B // NCORES     # 512 rows of label logits per core
WSCALE = 64.0         # fp8 pre-scale for w (subnormal-range fix), undone in exp

XA = 816              # exp cols on ScalarE (accum_out row-sum) = psA width
XD = CSH - XA         # fast-exp cols on VectorE (512) = psB width
XDH = XD // 2         # 256: half width for the fused fold+reduce

# Schraudolph fast-exp in f16: bits of exp(l) ~ round(l * 2^10/ln2 + M0H).
# M0H tuned for zero mean bias of sum(exp) under the empirical logit
# distribution (std ~0.303); K1S folds in the 1/WSCALE PSUM pre-scale.
K1H = 1477.3197218702985
M0H = 15301.739746
K1S = float(np.float32(K1H / WSCALE))

BF16 = mybir.dt.bfloat16
F16 = mybir.dt.float16
FP8 = mybir.dt.float8e4
F32 = mybir.dt.float32
I16 = mybir.dt.int16

_CACHE = {}


def _build_nc():
    nc = bacc.Bacc("TRN2", debug=False, target_bir_lowering=False)

    # critical pack: [wS_k0|fT0_k0 | wS_k1|fT0_k1] per partition
    HK = 2 * CSH + 1024
    H1 = KP * HK
    head1 = nc.dram_tensor("head1", [128, H1], FP8, kind="ExternalInput").ap()
    fTr = nc.dram_tensor("fTr", [KP, 128, 2, B - 512], FP8, kind="ExternalInput").ap()
    outS = nc.dram_tensor("outS", [128, 2 * BT], F32, kind="ExternalOutput").ap()

    with tile.TileContext(nc) as tc, ExitStack() as ctx:
        consts = ctx.enter_context(tc.tile_pool(name="consts", bufs=1))
        psums_a = ctx.enter_context(tc.tile_pool(name="psums_a", bufs=2, space="PSUM"))
        psums_b = ctx.enter_context(tc.tile_pool(name="psums_b", bufs=2, space="PSUM"))
        psumw = ctx.enter_context(tc.tile_pool(name="psumw", bufs=1, space="PSUM"))
        epool = ctx.enter_context(tc.tile_pool(name="epool", bufs=3))
        hpool = ctx.enter_context(tc.tile_pool(name="hpool", bufs=2))
        ypool = ctx.enter_context(tc.tile_pool(name="ypool", bufs=2))
        outs = ctx.enter_context(tc.tile_pool(name="outs", bufs=1))

        # input DMAs first, chained on the one SP HWDGE ring in priority
        # order: the queue drains in issue order, so head-A (everything
        # strip-0's k0 matmuls need) hits the wire first.
        CUT_A = 2 * CW[0] + 1024          # wS_k0_c0 + fT0_k0
        CUT_B = CUT_A + 2 * (CW[1] + CW[2])   # + wS_k0_c1, wS_k0_c2 (k0 half end)
        OFF_K0 = {0: 0, 1: CUT_A, 2: CUT_A + 2 * CW[1]}
        OFF_K1 = {0: CUT_B, 1: CUT_B + 2 * CW[0],
                  2: CUT_B + 2 * (CW[0] + CW[1])}
        FT1_OFF = CUT_B + 2 * CSH
        FSPLIT = 1024                     # fTr chunk split (strips 4-11 | 12-31)

        head1_sb = consts.tile([128, H1], FP8, tag="head1")
        nc.sync.dma_start(out=head1_sb[:, 0:CUT_B], in_=head1[:, 0:CUT_B])
        nc.sync.dma_start(out=head1_sb[:, CUT_B:H1], in_=head1[:, CUT_B:H1])
        fTr_sb = []
        for k in range(KP):
            t = consts.tile([128, 2, B - 512], FP8, tag=f"fTr{k}")
            fTr_sb.append(t)
        for k in range(KP):
            nc.sync.dma_start(
                out=fTr_sb[k][:, :, 0:FSPLIT], in_=fTr[k][:, :, 0:FSPLIT])
        for k in range(KP):
            nc.sync.dma_start(
                out=fTr_sb[k][:, :, FSPLIT:B - 512],
                in_=fTr[k][:, :, FSPLIT:B - 512])

        # PE HAM warm-up
        warm = consts.tile([128, 384], BF16, tag="warm")
        nc.gpsimd.memset(warm[:], 0.0)
        pwt = psumw.tile([128, 384], F32, tag="pw")
        for _ in range(12):
            nc.tensor.matmul(out=pwt[:], lhsT=warm[:, 0:128],
                             rhs=warm[:], start=True, stop=True)
        eprime = consts.tile([128, 1], F16, tag="eprime")
        nc.scalar.activation(
            out=eprime[:], in_=warm[:, 0:1].bitcast(F16),
            func=mybir.ActivationFunctionType.Exp, scale=1.0,
        )

        wS_sb = [
            {c: head1_sb[:, off:off + 2 * CW[c]].rearrange(
                "p (i n) -> p i n", i=2)
             for c, off in offs.items()}
            for offs in (OFF_K0, OFF_K1)
        ]
        fT0_sb = [
            head1_sb[:, 2 * 512:CUT_A].rearrange("p (i n) -> p i n", i=2),
            head1_sb[:, FT1_OFF:FT1_OFF + 1024].rearrange(
                "p (i n) -> p i n", i=2),
        ]

        # main GEMM; per-strip exp+row-sum split BY COLUMN across engines:
        # ScalarE exp+accum over [0:XA), VectorE f16 fast-exp + fused
        # fold+accum over [XA:CSH)
        st = outs.tile([128, 2 * BT], F32, tag="st")

        def emit_matmuls(b, psa, psb, k):
            for c in range(len(CW)):
                lhsT = (fT0_sb[k][:, :, b * 128:(b + 1) * 128]
                        if b < 4 else
                        fTr_sb[k][:, :, (b - 4) * 128:(b - 3) * 128])
                out_ap = (psa[:, CO[c]:CO[c] + CW[c]] if c < 2
                          else psb[:, 0:CW[2]])
                nc.tensor.matmul(
                    out=out_ap,
                    lhsT=lhsT,
                    rhs=wS_sb[k][c][:],
                    start=(k == 0),
                    stop=(k == KP - 1),
                    perf_mode=mybir.MatmulPerfMode.DoubleRow,
                )

        def emit_consumers(b, psa, psb):
            # ScalarE: exp over [0:XA) with the accumulator doing the row-sum
            e = epool.tile([128, XA], F16, tag="e")
            nc.scalar.activation(
                out=e[:], in_=psa[:],
                func=mybir.ActivationFunctionType.Exp,
                scale=float(1.0 / WSCALE),
                accum_out=st[:, b:b + 1],
            )
            # VectorE: f16 fast-exp over [XA:CSH), then one fused fold+accum
            y = ypool.tile([128, XD], I16, tag="y")
            nc.vector.tensor_scalar(
                out=y[:], in0=psb[:], scalar1=K1S, scalar2=float(M0H),
                op0=mybir.AluOpType.mult, op1=mybir.AluOpType.add,
            )
            hh = hpool.tile([128, XDH], F16, tag="h1")
            nc.vector.scalar_tensor_tensor(
                out=hh[:], in0=y[:, 0:XDH].bitcast(F16), scalar=1.0,
                in1=y[:, XDH:XD].bitcast(F16),
                op0=mybir.AluOpType.mult, op1=mybir.AluOpType.add,
                accum_out=st[:, BT + b:BT + b + 1],
            )

        # strips 0-1: both k0 pair-halves emitted before the k1 halves, so
        # the in-order PE can chew through head-A work while head-B is still
        # on the wire
        psa0 = psums_a.tile([128, XA], F32, tag="psa")
        psb0 = psums_b.tile([128, XD], F32, tag="psb")
        psa1 = psums_a.tile([128, XA], F32, tag="psa")
        psb1 = psums_b.tile([128, XD], F32, tag="psb")
        pro = {0: (psa0, psb0), 1: (psa1, psb1)}
        for k, bb in ((0, 0), (0, 1), (1, 0), (1, 1)):
            emit_matmuls(bb, pro[bb][0], pro[bb][1], k)
        for bb in (0, 1):
            emit_consumers(bb, pro[bb][0], pro[bb][1])

        for b in range(2, BT):
            psa = psums_a.tile([128, XA], F32, tag="psa")
            psb = psums_b.tile([128, XD], F32, tag="psb")
            for k in range(KP):
                emit_matmuls(b, psa, psb, k)
            emit_consumers(b, psa, psb)
            if b == 15:
                nc.sync.dma_start(out=outS[:, 0:16], in_=st[:, 0:16])
            if b == BT - 4:
                nc.sync.dma_start(
                    out=outS[:, 16:BT - 4], in_=st[:, 16:BT - 4])
        # final chunk from the Activation ring: the last ACT accums plus the
        # whole fast-exp plane
        nc.scalar.dma_start(out=outS[:, BT - 4:], in_=st[:, BT - 4:])

    nc.compile()
    return nc


def _core_sizes():
    sizes = [CS_BASE] * (NCORES - 1) + [C - CS_BASE * (NCORES - 1)]
    starts = np.concatenate([[0], np.cumsum(sizes)[:-1]]).astype(np.int64)
    return np.array(sizes, dtype=np.int64), starts


def _prepare_inputs(feats, labels, w):
    sizes, starts = _core_sizes()

    in_maps = []
    fp8_feats = feats.astype(ml_dtypes.float8_e4m3)
    for p in range(NCORES):
        frolled = np.roll(fp8_feats, -p * BSH, axis=0)
        fT_host = np.ascontiguousarray(
            frolled.reshape(B, KP, 2, 128).transpose(1, 3, 2, 0)
        )
        fTr_host = np.ascontiguousarray(fT_host[:, :, :, 512:])
        c0, sz = int(starts[p]), int(sizes[p])
        wp = np.zeros((D, CSH), dtype=np.float32)
        wp[:, :sz] = w[:, c0:c0 + sz] * WSCALE
        wS_host = np.ascontiguousarray(
            wp.reshape(KP, 2, 128, CSH).transpose(0, 2, 1, 3)
        ).astype(ml_dtypes.float8_e4m3)

        def ctile(k, c):
            return np.ascontiguousarray(
                wS_host[k][:, :, CO[c]:CO[c] + CW[c]]).reshape(128, -1)

        head1_host = np.concatenate(
            [ctile(0, 0),
             np.ascontiguousarray(fT_host[0][:, :, 0:512]).reshape(128, -1),
             ctile(0, 1), ctile(0, 2),
             ctile(1, 0), ctile(1, 1), ctile(1, 2),
             np.ascontiguousarray(fT_host[1][:, :, 0:512]).reshape(128, -1)],
            axis=1,
        )
        in_maps.append({
            "head1": np.ascontiguousarray(head1_host),
            "fTr": fTr_host,
        })
    return in_maps


def _run(in_maps, trace=False):
    if "nc" not in _CACHE:
        _CACHE["nc"] = _build_nc()
    nc = _CACHE["nc"]
    return run_bass_kernel_spmd(
        nc, in_maps, core_ids=list(range(NCORES)), trace=trace
    )


def _fastexp_host(ps_vals):
    """Replica of the device f16 fast-exp for f32 PSUM values:
    f16 bits = rint(f32(f32(ps * K1S) + M0H)), read back as f16 floats."""
    x = np.asarray(ps_vals, dtype=np.float32)
    y = np.float32(x * np.float32(K1S)) + np.float32(M0H)
    return np.rint(y).astype(np.int16).view(np.float16).astype(np.float64)


def kernel(feats, labels, centers, counts, w, _trace=False, _ret_res=False):
    feats = np.asarray(feats, dtype=np.float32)
    labels_i = np.asarray(labels).astype(np.int64)
    centers = np.asarray(centers, dtype=np.float32)
    counts = np.asarray(counts, dtype=np.float32)
    w = np.asarray(w, dtype=np.float32)

    in_maps = _prepare_inputs(feats, labels_i, w)
    res = _run(in_maps, trace=_trace)

    sizes, starts = _core_sizes()

    means = (centers / counts[:, None]).astype(np.float32)
    nrm = np.sqrt((means.astype(np.float32) ** 2).sum(axis=1, keepdims=True))
    mn = (means / nrm).astype(np.float32)
    dsq = (mn.astype(np.float64) ** 2).sum(axis=1)       # [C]
    d = dsq[labels_i]                                    # [B]

    f8 = feats.astype(ml_dtypes.float8_e4m3).astype(np.float64)      # [B, D]
    w8 = (w * WSCALE).astype(ml_dtypes.float8_e4m3).astype(np.float64)
    t_ps = np.einsum("bd,bd->b", f8, w8[:, labels_i].T)              # [B]
    t = t_ps / WSCALE

    # pad cols (local idx >= sizes[p] >= XA) all take the fast-exp path:
    # each contributes fastexp16(0) per strip row
    fastexp0 = float(_fastexp_host(np.zeros(1))[0])

    S_tot = np.zeros(B, dtype=np.float64)
    for p in range(NCORES):
        # outS[q, b] is rolled row b*128 + q = original row (b*128+q+p*BSH)%B
        sp = res.results[p]["outS"].astype(np.float64)   # [128, 2*BT]
        sb = sp[:, :BT] + sp[:, BT:]                     # ACT + DVE shares
        S_p = sb.T.reshape(B)                            # rolled rows
        pad_p = float(CSH - sizes[p])
        S_p = S_p - pad_p * fastexp0
        S_tot += np.roll(S_p, p * BSH)

    # subtract the device's own label-column contribution: row i's label
    # class lives in shard p*, in the ACT range if its local column < XA
    p_star = np.minimum(labels_i // CS_BASE, NCORES - 1)
    c_local = labels_i - starts[p_star]
    lab_dev = np.where(
        c_local < XA,
        np.exp(t),
        _fastexp_host(t_ps),
    )
    z = S_tot - lab_dev + np.exp(t + d)
    nll = np.log(z) - (t + d)
    loss = np.float32(nll.mean())
    out = np.array(loss, dtype=np.float32)
    if _ret_res:
        return out, res
    return out
